# revision 9
# baseline (speedup 1.0000x reference)
"""Trainium2 Bass kernel for nn_ACPClassifier (RetNet-style block + classifier head).

Strategy:
- Only the last vocab iteration of the reference affects the output (x is
  overwritten each pass), so we compute a single block on emb_tables[2]/ids[2].
- Data-parallel over batch: 8 batch elements -> 8 NeuronCores, no collectives.
- d-major activation layout [128 part, 4 dblk, 1024 tok]; retention decay
  gamma^(n-m) factorized into gamma^n * q and gamma^-m * k (folded into the
  PSUM->SBUF copies of the q/k projections); scores materialized triangularly
  (lower key-blocks only) with a causal mask on the diagonal blocks.
- fp32r matmuls for projections / norms / stats, bf16 for attention operands.
"""

import numpy as np

import concourse.bacc as bacc
import concourse.mybir as mybir
from concourse import bass
from concourse.bass_utils import run_bass_kernel_spmd
from concourse.tile import TileContext

F32 = mybir.dt.float32
FR = mybir.dt.float32r
BF = mybir.dt.bfloat16
AF = mybir.ActivationFunctionType
OP = mybir.AluOpType

DIM, SEQ, HEADS, HDIM, BATCH, VOCAB, NVOCAB = 512, 1024, 8, 64, 8, 1024, 3
EPS = 1e-5
P = 128
NDB = DIM // P   # 4 d-blocks
NTB = SEQ // P   # 8 token blocks
NCH = SEQ // 512  # 2 free-dim chunks of 512
NCORES = 8


def _fr(ap):
    return ap.bitcast(FR)


def _st_mm_layout(j):
    """Score-block MM chunks for key-block j: list of (chunk, col_start, width).

    Covers columns [j*128, 1024) with bank-aligned matmuls of width >= 256
    (fp32r/bf16 full rate); width-128 tails are widened left by 128 (the extra
    columns are garbage that the SBUF copies never read).
    """
    out = []
    c0 = (j * P) // 512
    for c in range(c0, NCH):
        s = max(j * P, c * 512)
        w = (c + 1) * 512 - s
        if w == 128:
            s -= 128
            w = 256
        out.append((c, s, w))
    return out


def build_nc():
    nc = bacc.Bacc(
        "TRN2",
        target_bir_lowering=False,
        debug=False,
        enable_asserts=False,
        num_devices=NCORES,
    )

    # ---- DRAM parameters (per-core inputs) ----
    d_emb = nc.declare_dram_parameter("embT", [P, NDB, SEQ], FR, isOutput=False)
    d_w = {
        k: nc.declare_dram_parameter(k, [P, NDB, DIM], FR, isOutput=False)
        for k in ("wq", "wk", "wv", "wg", "wo", "w1", "w2")
    }
    d_dq = nc.declare_dram_parameter("dqc", [P, NDB, SEQ], F32, isOutput=False)
    d_dk = nc.declare_dram_parameter("dkc", [P, NDB, SEQ], F32, isOutput=False)
    d_mask = nc.declare_dram_parameter("mask01", [P, P], F32, isOutput=False)
    d_sel8 = nc.declare_dram_parameter("sel8", [P, NDB, HEADS], FR, isOutput=False)
    d_selb = nc.declare_dram_parameter("selB", [HEADS, NDB, P], FR, isOutput=False)
    d_gam = nc.declare_dram_parameter("gam", [P, NDB], F32, isOutput=False)
    d_bet = nc.declare_dram_parameter("bet", [P, NDB], F32, isOutput=False)
    d_b1 = nc.declare_dram_parameter("b1t", [P, NDB], F32, isOutput=False)
    d_b2 = nc.declare_dram_parameter("b2t", [P, NDB], F32, isOutput=False)
    d_fct = nc.declare_dram_parameter("fcT", [P, NDB, SEQ], F32, isOutput=False)
    d_fcb = nc.declare_dram_parameter("fcb", [1, 1], F32, isOutput=False)
    d_ones1 = nc.declare_dram_parameter("ones1", [P, 1], FR, isOutput=False)
    d_onesb = nc.declare_dram_parameter("onesb", [1, P], FR, isOutput=False)
    d_out = nc.declare_dram_parameter("out", [1, 1], F32, isOutput=True)

    with TileContext(nc) as tc:
        from contextlib import ExitStack

        ctx = ExitStack()
        with ctx:
            acts = ctx.enter_context(tc.tile_pool(name="acts", bufs=1))
            wts = ctx.enter_context(tc.tile_pool(name="wts", bufs=2))
            smal = ctx.enter_context(tc.tile_pool(name="smal", bufs=1))
            pp = ctx.enter_context(tc.tile_pool(name="pp", bufs=1, space="PSUM"))

            # ---- constants ----
            mask01 = smal.tile([P, P], F32, name="mask01")
            nc.sync.dma_start(mask01[:], d_mask[:])
            sel8 = smal.tile([P, NDB, HEADS], FR, name="sel8")
            nc.sync.dma_start(sel8[:], d_sel8[:])
            selB = smal.tile([HEADS, NDB, P], FR, name="selB")
            nc.sync.dma_start(selB[:], d_selb[:])
            gam = smal.tile([P, NDB], F32, name="gam")
            nc.sync.dma_start(gam[:], d_gam[:])
            bet = smal.tile([P, NDB], F32, name="bet")
            nc.sync.dma_start(bet[:], d_bet[:])
            b1t = smal.tile([P, NDB], F32, name="b1t")
            nc.sync.dma_start(b1t[:], d_b1[:])
            b2t = smal.tile([P, NDB], F32, name="b2t")
            nc.sync.dma_start(b2t[:], d_b2[:])
            fcb = smal.tile([1, 1], F32, name="fcb")
            nc.sync.dma_start(fcb[:], d_fcb[:])
            ones1 = smal.tile([P, 1], FR, name="ones1")
            nc.sync.dma_start(ones1[:], d_ones1[:])
            onesb = smal.tile([1, P], FR, name="onesb")
            nc.sync.dma_start(onesb[:], d_onesb[:])
            onesf = smal.tile([P, 1], F32, name="onesf")
            nc.gpsimd.memset(onesf[:], 1.0)

            # ---- big activation tiles ----
            xe = acts.tile([P, NDB, SEQ], FR, tag="t_emb", name="xe")  # emb -> xhat1
            nc.sync.dma_start(xe[:], d_emb[:])
            dqc = acts.tile([P, NDB, SEQ], F32, tag="t_gnm", name="dqc")
            nc.sync.dma_start(dqc[:], d_dq[:])
            dkc = acts.tile([P, NDB, SEQ], F32, tag="t_gnr", name="dkc")
            nc.sync.dma_start(dkc[:], d_dk[:])

            def load_w(key):
                t = wts.tile([P, NDB, DIM], FR, tag="t_w", name=f"w_{key}")
                nc.sync.dma_start(t[:], d_w[key][:])
                return t

            # ---------------- layer norm helper ----------------
            def layer_norm(src, dst, sq_tag, sq_name, scope):
                """dst = LN(src) with gamma/beta. src/dst may be the same tile."""
                with nc.named_scope(scope):
                    sq = acts.tile([P, NDB, SEQ], FR, tag=sq_tag, name=sq_name)
                    for c in range(NDB):
                        nc.gpsimd.tensor_tensor(
                            sq[:, c], src[:, c], src[:, c], OP.mult
                        )
                    # per-token sums via ones-matmul (contract partition dim)
                    s_ps, ss_ps = [], []
                    for which, rhs_t in ((0, src), (1, sq)):
                        for ch in range(NCH):
                            ps = pp.tile(
                                [1, 512], F32, tag="mm", bufs=5,
                                name=f"{sq_name}_s{which}{ch}",
                            )
                            for kb in range(NDB):
                                nc.tensor.matmul(
                                    ps[:],
                                    lhsT=ones1[:],
                                    rhs=rhs_t[:, kb, ch * 512:(ch + 1) * 512],
                                    start=(kb == 0),
                                    stop=(kb == NDB - 1),
                                )
                            (s_ps if which == 0 else ss_ps).append(ps)
                    # rows to SBUF
                    srow = acts.tile([1, SEQ], FR, tag="t_rows", bufs=4,
                                     name=f"{sq_name}_srow")
                    ssrow = acts.tile([1, SEQ], FR, tag="t_rows", bufs=4,
                                      name=f"{sq_name}_ssrow")
                    for ch in range(NCH):
                        sl = slice(ch * 512, ch * 512 + 512)
                        nc.scalar.copy(srow[:, sl], s_ps[ch][:])
                        nc.scalar.copy(ssrow[:, sl], ss_ps[ch][:])
                    # broadcast raw sums to 128 partitions
                    negmu = acts.tile([P, SEQ], F32, tag="t_nmu", bufs=2,
                                      name=f"{sq_name}_nmu")
                    rstd = acts.tile([P, SEQ], F32, tag="t_nrs", bufs=2,
                                     name=f"{sq_name}_nrs")
                    tmpb = acts.tile([P, SEQ], F32, tag="t_tmpb", bufs=1,
                                     name=f"{sq_name}_tmpb")
                    for ch in range(NCH):
                        sl = slice(ch * 512, ch * 512 + 512)
                        bs = pp.tile([P, 512], F32, tag="mm", bufs=5,
                                     name=f"{sq_name}_bs{ch}")
                        nc.tensor.matmul(
                            bs[:], lhsT=onesb[:], rhs=srow[:, sl],
                            start=True, stop=True,
                        )
                        bss = pp.tile([P, 512], F32, tag="mm", bufs=5,
                                      name=f"{sq_name}_bss{ch}")
                        nc.tensor.matmul(
                            bss[:], lhsT=onesb[:], rhs=ssrow[:, sl],
                            start=True, stop=True,
                        )
                        # negmu = -sum/D ; rstd = 1/sqrt(sumsq/D - mu^2 + eps)
                        nc.scalar.activation(negmu[:, sl], bs[:], AF.Copy,
                                             scale=-1.0 / DIM)
                        nc.scalar.activation(tmpb[:, sl], bs[:], AF.Square,
                                             scale=1.0 / DIM)
                        nc.vector.tensor_scalar(rstd[:, sl], bss[:], 1.0 / DIM,
                                                EPS, OP.mult, OP.add)
                        nc.vector.tensor_sub(rstd[:, sl], rstd[:, sl], tmpb[:, sl])
                        nc.vector.reciprocal(rstd[:, sl], rstd[:, sl])
                        nc.scalar.activation(rstd[:, sl], rstd[:, sl], AF.Sqrt)
                    # apply
                    for c in range(NDB):
                        nc.gpsimd.tensor_tensor(dst[:, c], src[:, c], negmu[:],
                                                OP.add)
                        nc.vector.tensor_tensor(dst[:, c], dst[:, c], rstd[:],
                                                OP.mult)
                        nc.vector.tensor_scalar(dst[:, c], dst[:, c],
                                                gam[:, c:c + 1], bet[:, c:c + 1],
                                                OP.mult, OP.add)

            # ---------------- d-major projection helper ----------------
            def proj_dmajor(w_tile, src, writer, scope):
                with nc.named_scope(scope):
                    for mb in range(NDB):
                        for ch in range(NCH):
                            ps = pp.tile([P, 512], F32, tag="mm", bufs=5,
                                         name=f"{scope}_ps{mb}{ch}")
                            for kb in range(NDB):
                                nc.tensor.matmul(
                                    ps[:],
                                    lhsT=w_tile[:, kb, mb * P:(mb + 1) * P],
                                    rhs=src[:, kb, ch * 512:(ch + 1) * 512],
                                    start=(kb == 0),
                                    stop=(kb == NDB - 1),
                                )
                            writer(mb, ch, ps)

            # ================= phase 1: LN1 =================
            layer_norm(xe, xe, "t_sq", "sq1", "ln1")
            xh1 = xe  # LN1 output (in-place)

            # ================= phase 2: q/k/g/v projections =================
            qt = acts.tile([P, NDB, SEQ], BF, tag="t_q", name="qt")
            kt = acts.tile([P, NDB, SEQ], BF, tag="t_k", name="kt")
            sw = acts.tile([P, NDB, SEQ], F32, tag="t_sw", name="sw")
            vt = acts.tile([P, NTB, DIM], BF, tag="t_v", name="vt")

            wq_t = load_w("wq")

            def wr_q(mb, ch, ps):
                sl = slice(ch * 512, ch * 512 + 512)
                nc.vector.tensor_tensor(qt[:, mb, sl], ps[:], dqc[:, mb, sl],
                                        OP.mult)

            proj_dmajor(wq_t, xh1, wr_q, "proj_q")

            wk_t = load_w("wk")

            def wr_k(mb, ch, ps):
                sl = slice(ch * 512, ch * 512 + 512)
                nc.vector.tensor_tensor(kt[:, mb, sl], ps[:], dkc[:, mb, sl],
                                        OP.mult)

            proj_dmajor(wk_t, xh1, wr_k, "proj_k")

            wg_t = load_w("wg")

            def wr_g(mb, ch, ps):
                sl = slice(ch * 512, ch * 512 + 512)
                # swish(x) = x * sigmoid(x); CoreSim lacks Silu so build it
                nc.scalar.activation(sw[:, mb, sl], ps[:], AF.Sigmoid)
                nc.vector.tensor_tensor(sw[:, mb, sl], sw[:, mb, sl], ps[:],
                                        OP.mult)

            proj_dmajor(wg_t, xh1, wr_g, "proj_g")

            wv_t = load_w("wv")
            with nc.named_scope("proj_v"):
                for tb in range(NTB):
                    ps = pp.tile([P, 512], F32, tag="mm", bufs=5,
                                 name=f"vps{tb}")
                    for kb in range(NDB):
                        nc.tensor.matmul(
                            ps[:],
                            lhsT=xh1[:, kb, tb * P:(tb + 1) * P],
                            rhs=wv_t[:, kb, :],
                            start=(kb == 0),
                            stop=(kb == NDB - 1),
                        )
                    nc.scalar.copy(vt[:, tb, :], ps[:])

            # ================= phase 3: retention attention =================
            y = acts.tile([P, NDB, SEQ], FR, tag="t_y", name="y")
            ysq = acts.tile([P, NDB, SEQ], FR, tag="t_sq", name="ysq")

            with nc.named_scope("attn"):
                for t in range(NDB):  # head pair (2t, 2t+1) lives in d-block t
                    yt_ps = [
                        pp.tile([P, 512], F32, tag="yt", bufs=2,
                                name=f"yt{t}_{ch}")
                        for ch in range(NCH)
                    ]
                    for j in range(NTB):
                        for r in (0, 64):
                            h = 2 * t + (r // 64)
                            st_sb = acts.tile([P, SEQ], BF, tag="t_st", bufs=4,
                                              name=f"st{h}_{j}")
                            ps_by_c = {}
                            for (c, s, w) in _st_mm_layout(j):
                                ps = pp.tile([P, 512], F32, tag="mm", bufs=5,
                                             name=f"stp{h}_{j}_{c}")
                                nc.tensor.matmul(
                                    ps[:, s - c * 512: s - c * 512 + w],
                                    lhsT=kt[r:r + 64, t, j * P:(j + 1) * P],
                                    rhs=qt[r:r + 64, t, s:s + w],
                                    start=True,
                                    stop=True,
                                )
                                ps_by_c[c] = ps
                            # copies PSUM -> SBUF (bf16), masking the diagonal
                            c0 = (j * P) // 512
                            dof = j * P - c0 * 512
                            nc.vector.tensor_tensor(
                                st_sb[:, j * P:(j + 1) * P],
                                ps_by_c[c0][:, dof:dof + P],
                                mask01[:],
                                OP.mult,
                            )
                            if dof + P < 512:
                                nc.scalar.copy(
                                    st_sb[:, (j + 1) * P:(c0 + 1) * 512],
                                    ps_by_c[c0][:, dof + P:512],
                                )
                            for c in range(c0 + 1, NCH):
                                nc.scalar.copy(
                                    st_sb[:, c * 512:(c + 1) * 512],
                                    ps_by_c[c][:],
                                )
                            # AV contributions of key-block j
                            for ch in range(NCH):
                                hi = 4 * ch + 3
                                if j > hi:
                                    continue
                                s_av = max(j * P, ch * 512)
                                w_av = (hi + 1) * P - s_av
                                # the two heads of a pair col-tile into the
                                # same PSUM bank on disjoint partition halves;
                                # the per-bank group check is conservative here
                                nc.tensor.matmul(
                                    yt_ps[ch][r:r + 64,
                                              s_av - ch * 512:
                                              s_av - ch * 512 + w_av],
                                    lhsT=vt[:, j, h * HDIM:(h + 1) * HDIM],
                                    rhs=st_sb[:, s_av:s_av + w_av],
                                    start=(j == 0),
                                    stop=(j == hi),
                                    tile_position=(0, r),
                                    skip_group_check=True,
                                )
                    # y and y^2 out of PSUM
                    for ch in range(NCH):
                        sl = slice(ch * 512, ch * 512 + 512)
                        nc.scalar.copy(y[:, t, sl], yt_ps[ch][:])
                        nc.scalar.activation(ysq[:, t, sl], yt_ps[ch][:],
                                             AF.Square)

            # ---- group norm (per head, per token) + gate ----
            with nc.named_scope("gnorm"):
                grow_ps, gssrow_ps = [], []
                for which, src_t in ((0, y), (1, ysq)):
                    for ch in range(NCH):
                        ps = pp.tile([HEADS, 512], F32, tag="mm", bufs=5,
                                     name=f"gn{which}{ch}")
                        for c in range(NDB):
                            nc.tensor.matmul(
                                ps[:],
                                lhsT=sel8[:, c, :],
                                rhs=src_t[:, c, ch * 512:(ch + 1) * 512],
                                start=(c == 0),
                                stop=(c == NDB - 1),
                            )
                        (grow_ps if which == 0 else gssrow_ps).append(ps)
                negmu8 = acts.tile([HEADS, SEQ], FR, tag="t_rows", bufs=4,
                                   name="negmu8")
                rstd8 = acts.tile([HEADS, SEQ], FR, tag="t_rows", bufs=4,
                                  name="rstd8")
                tmp8 = acts.tile([HEADS, SEQ], F32, tag="t_rows", bufs=4,
                                 name="tmp8")
                var8 = acts.tile([HEADS, SEQ], F32, tag="t_rows", bufs=4,
                                 name="var8")
                for ch in range(NCH):
                    sl = slice(ch * 512, ch * 512 + 512)
                    nc.scalar.activation(negmu8[:, sl], grow_ps[ch][:], AF.Copy,
                                         scale=-1.0 / HDIM)
                    nc.scalar.activation(tmp8[:, sl], grow_ps[ch][:], AF.Square,
                                         scale=1.0 / HDIM)
                    nc.vector.tensor_scalar(var8[:, sl], gssrow_ps[ch][:],
                                            1.0 / HDIM, EPS, OP.mult, OP.add)
                    nc.vector.tensor_sub(var8[:, sl], var8[:, sl], tmp8[:, sl])
                    nc.vector.reciprocal(var8[:, sl], var8[:, sl])
                    nc.scalar.activation(rstd8[:, sl], var8[:, sl], AF.Sqrt)
                # broadcast to full tensors (reuse dq/dk slots)
                gnm = acts.tile([P, NDB, SEQ], F32, tag="t_gnm", name="gnm")
                gnr = acts.tile([P, NDB, SEQ], F32, tag="t_gnr", name="gnr")
                for c in range(NDB):
                    for ch in range(NCH):
                        sl = slice(ch * 512, ch * 512 + 512)
                        bm = pp.tile([P, 512], F32, tag="mm", bufs=5,
                                     name=f"gbm{c}{ch}")
                        nc.tensor.matmul(bm[:], lhsT=selB[:, c, :],
                                         rhs=negmu8[:, sl],
                                         start=True, stop=True)
                        nc.scalar.copy(gnm[:, c, sl], bm[:])
                        br = pp.tile([P, 512], F32, tag="mm", bufs=5,
                                     name=f"gbr{c}{ch}")
                        nc.tensor.matmul(br[:], lhsT=selB[:, c, :],
                                         rhs=rstd8[:, sl],
                                         start=True, stop=True)
                        nc.scalar.copy(gnr[:, c, sl], br[:])
                # apply GN + gate in place on y
                for c in range(NDB):
                    nc.gpsimd.tensor_tensor(y[:, c], y[:, c], gnm[:, c], OP.add)
                    nc.vector.tensor_tensor(y[:, c], y[:, c], gnr[:, c], OP.mult)
                    nc.gpsimd.tensor_tensor(y[:, c], y[:, c], sw[:, c], OP.mult)

            # ================= phase 4: output proj + residual =================
            x1 = acts.tile([P, NDB, SEQ], FR, tag="t_k", name="x1")
            wo_t = load_w("wo")

            def wr_o(mb, ch, ps):
                sl = slice(ch * 512, ch * 512 + 512)
                nc.vector.tensor_tensor(x1[:, mb, sl], ps[:], xh1[:, mb, sl],
                                        OP.add)

            proj_dmajor(wo_t, y, wr_o, "proj_o")

            # ================= phase 5: LN2 =================
            xh2 = acts.tile([P, NDB, SEQ], FR, tag="t_q", name="xh2")
            layer_norm(x1, xh2, "t_y", "sq2", "ln2")

            # ================= phase 6: FFN =================
            h1 = acts.tile([P, NDB, SEQ], FR, tag="t_emb", name="h1")
            w1_t = load_w("w1")

            def wr_h1(mb, ch, ps):
                sl = slice(ch * 512, ch * 512 + 512)
                nc.scalar.activation(h1[:, mb, sl], ps[:], AF.Relu,
                                     bias=b1t[:, mb:mb + 1])

            proj_dmajor(w1_t, xh2, wr_h1, "ffn1")

            x2 = acts.tile([P, NDB, SEQ], FR, tag="t_y", name="x2")
            w2_t = load_w("w2")

            def wr_x2(mb, ch, ps):
                sl = slice(ch * 512, ch * 512 + 512)
                nc.vector.tensor_tensor(x2[:, mb, sl], ps[:], x1[:, mb, sl],
                                        OP.add)
                nc.gpsimd.tensor_scalar(x2[:, mb, sl], x2[:, mb, sl],
                                        b2t[:, mb:mb + 1], None, OP.add)

            proj_dmajor(w2_t, h1, wr_x2, "ffn2")

            # ================= phase 7: LN3 =================
            xf = acts.tile([P, NDB, SEQ], F32, tag="t_q", name="xf")
            layer_norm(x2, xf, "t_sq", "sq3", "ln3")

            # ================= phase 8: classifier =================
            with nc.named_scope("classifier"):
                fct = acts.tile([P, NDB, SEQ], F32, tag="t_emb", name="fct")
                nc.sync.dma_start(fct[:], d_fct[:])
                z = acts.tile([P, NDB, SEQ], F32, tag="t_sw", name="z")
                zred = smal.tile([P, 1], F32, name="zred")
                nc.vector.tensor_mul(z[:], xf[:], fct[:])
                nc.vector.tensor_reduce(zred[:], z[:],
                                        axis=mybir.AxisListType.XY, op=OP.add)
                lg_ps = pp.tile([1, 512], F32, tag="mm", bufs=5, name="lg")
                nc.tensor.matmul(lg_ps[0:1, 0:1], lhsT=onesf[:],
                                 rhs=zred[:], start=True, stop=True)
                prob = smal.tile([1, 1], F32, name="prob")
                nc.scalar.activation(prob[:], lg_ps[0:1, 0:1], AF.Sigmoid,
                                     bias=fcb[:])
                nc.sync.dma_start(d_out[:], prob[:])

    nc.finalize()
    return nc


# ---------------- host-side input prep ----------------

def prep_in_maps(inputs):
    ids = np.asarray(inputs["ids"])[NVOCAB - 1]          # [B, S]
    tab = np.asarray(inputs["emb_tables"], np.float32)[NVOCAB - 1]   # [V, D]
    pos = np.asarray(inputs["pos_emb"], np.float32)[NVOCAB - 1]      # [S, D]

    def dmajor(a2d):  # [S, D] -> [P, NDB, S]
        return np.ascontiguousarray(
            a2d.T.reshape(NDB, P, SEQ).transpose(1, 0, 2), np.float32
        )

    def wblocks(w):  # [Din, Dout] -> [P, NDB, Dout]
        return np.ascontiguousarray(
            w.reshape(NDB, P, DIM).transpose(1, 0, 2), np.float32
        )

    def pvec(v):  # [D] -> [P, NDB]
        return np.ascontiguousarray(v.reshape(NDB, P).T, np.float32)

    gammas = 1.0 - 2.0 ** (-5.0 - np.arange(HEADS, dtype=np.float64))
    n = np.arange(SEQ, dtype=np.float64)
    # head of (partition p, block c) is 2c + p//64
    hmap = (2 * np.arange(NDB)[None, :] + (np.arange(P) // HDIM)[:, None])  # [P, NDB]
    lng = np.log(gammas)[hmap]                                   # [P, NDB]
    dqc = np.exp(lng[:, :, None] * n[None, None, :]) / np.sqrt(HDIM)
    dkc = np.exp(-lng[:, :, None] * n[None, None, :])
    dqc = np.ascontiguousarray(dqc, np.float32)
    dkc = np.ascontiguousarray(dkc, np.float32)

    mask01 = (n[None, :P] >= n[:P, None]).astype(np.float32)     # [b, a] a>=b
    sel8 = np.zeros((P, NDB, HEADS), np.float32)
    selB = np.zeros((HEADS, NDB, P), np.float32)
    for c in range(NDB):
        for p in range(P):
            sel8[p, c, 2 * c + p // HDIM] = 1.0
            selB[2 * c + p // HDIM, c, p] = 1.0

    fct = np.ascontiguousarray(
        np.asarray(inputs["fc_W"], np.float32).reshape(SEQ, DIM)
        .reshape(SEQ, NDB, P).transpose(2, 1, 0)
    )

    common = {
        "wq": wblocks(np.asarray(inputs["Wq"], np.float32)),
        "wk": wblocks(np.asarray(inputs["Wk"], np.float32)),
        "wv": wblocks(np.asarray(inputs["Wv"], np.float32)),
        "wg": wblocks(np.asarray(inputs["Wg"], np.float32)),
        "wo": wblocks(np.asarray(inputs["Wo"], np.float32)),
        "w1": wblocks(np.asarray(inputs["W1"], np.float32)),
        "w2": wblocks(np.asarray(inputs["W2"], np.float32)),
        "dqc": dqc,
        "dkc": dkc,
        "mask01": mask01,
        "sel8": sel8,
        "selB": selB,
        "gam": pvec(np.asarray(inputs["ln_gamma"], np.float32)),
        "bet": pvec(np.asarray(inputs["ln_beta"], np.float32)),
        "b1t": pvec(np.asarray(inputs["b1"], np.float32)),
        "b2t": pvec(np.asarray(inputs["b2"], np.float32)),
        "fcT": fct,
        "fcb": np.asarray(inputs["fc_b"], np.float32).reshape(1, 1),
        "ones1": np.ones((P, 1), np.float32),
        "onesb": np.ones((1, P), np.float32),
    }

    in_maps = []
    for b in range(BATCH):
        emb = tab[ids[b]] + pos                                  # [S, D]
        m = dict(common)
        m["embT"] = np.ascontiguousarray(
            emb.T.reshape(NDB, P, SEQ).transpose(1, 0, 2), np.float32
        )
        in_maps.append(m)
    return in_maps


_NC_CACHE = {}


def get_nc():
    if "nc" not in _NC_CACHE:
        _NC_CACHE["nc"] = build_nc()
    return _NC_CACHE["nc"]


def kernel(**inputs) -> np.ndarray:
    nc = get_nc()
    in_maps = prep_in_maps(inputs)
    res = run_bass_kernel_spmd(nc, in_maps, core_ids=list(range(NCORES)))
    out = np.stack(
        [np.asarray(res.results[b]["out"]).reshape(()) for b in range(BATCH)]
    ).reshape(BATCH, 1)
    return out.astype(np.float32)
